# revision 6
# baseline (speedup 1.0000x reference)
"""Trainium2 Bass kernel v6 for nn_Attention_Module.

Same algebraic rewrite as the baseline:
    qt[b, t] = sum_a Q[b, a] Wk[a, t];  cb[b] = sum_a Q[b, a] bk[a]
    e[b, l]  = (reviewer[b, l, :] . qt[b, :] + cb[b]) / sqrt(A)
    ww[b, t] = sum_l e[b, l] reviewer[b, l, t]

v6 vs the local-weights version: each core loads only a 128-row slice of
Wq/Wk (1.03 MiB instead of 8.4 MiB), computes the partial
qt[b, :] = sum_{a in slice} Q[b, a] Wk[a, :] for ALL 32 batches, and an
8-core ReduceScatter sums the partials so each core receives exactly the
qt rows of its own 4 batches.  A tiny warmup AllReduce is issued at
kernel start so ncfw's first-collective startup cost overlaps the DMA
ramp instead of sitting on the qt critical path.  Cutting 7.4 MiB of
weight DMA per core moves the reviewer-stream end in by ~17 us.

Sharding: data-parallel over batch B=32 -> 4 batches per core x 8 cores.
Host-side prep is layout-only (transpose/reshape/slice).
"""

import numpy as np

import concourse.bass as bass
import concourse.bacc as bacc
import concourse.tile as tile
from concourse.tile_rust import add_dep_helper
from concourse import mybir
from concourse.bass_utils import run_bass_kernel_spmd

B, L, T, A = 32, 2048, 1024, 1024
NCORES = 8
BLOC = B // NCORES
P = 128
NCH = T // P
LTILE = 4
ROWS_PER_TILE = P * LTILE
NTI = L // ROWS_PER_TILE
SCALE = 1.0 / float(np.sqrt(A))

F32 = mybir.dt.float32
FT = mybir.ActivationFunctionType
OP = mybir.AluOpType


def _build():
    nc = bacc.Bacc("TRN2", target_bir_lowering=False, debug=False, num_devices=NCORES)

    rev = nc.dram_tensor("rev", [BLOC, L, T], F32, kind="ExternalInput").ap()
    # subt[p, j, b] = submitter[b, 128j + p]  (ALL 32 batches)
    subt = nc.dram_tensor("subt", [P, NCH, B], F32, kind="ExternalInput").ap()
    # wqtc[p, j, a'] = Wq[128c + a', 128j + p]  (this core's a-slice)
    wqtc = nc.dram_tensor("wqtc", [P, NCH, P], F32, kind="ExternalInput").ap()
    # wkce[a', 0:1024] = Wk[128c + a', :]; col 1024 = bk[128c + a']; pad to 1028
    wkce = nc.dram_tensor("wkce", [P, 1028], F32, kind="ExternalInput").ap()
    ones1 = nc.dram_tensor("ones1", [1, B], F32, kind="ExternalInput").ap()
    bqc1 = nc.dram_tensor("bqc1", [1, P], F32, kind="ExternalInput").ap()
    # sel4[k, 128b + m] = (k == b)
    sel4 = nc.dram_tensor("sel4", [BLOC, BLOC * P], F32, kind="ExternalInput").ap()
    id32 = nc.dram_tensor("id32", [B, B], F32, kind="ExternalInput").ap()
    out = nc.dram_tensor("out", [BLOC, T], F32, kind="ExternalOutput").ap()

    with tile.TileContext(nc) as tc:
        with (
            tc.tile_pool(name="small", bufs=1) as small,
            tc.tile_pool(name="rp", bufs=7) as rp,
            tc.tile_pool(name="scr", bufs=3) as scrp,
            tc.tile_pool(name="ep", bufs=4) as ep,
            tc.tile_pool(name="wwp", bufs=2) as wwp,
            tc.tile_pool(name="dram", bufs=1, space="DRAM") as dram,
            tc.tile_pool(name="psq", bufs=1, space="PSUM") as psqp,
            tc.tile_pool(name="psb", bufs=2, space="PSUM") as psbp,
            tc.tile_pool(name="psww", bufs=2, space="PSUM") as pswwp,
        ):
            # ---- small loads (weights are tiny now: ~1.1 MiB total) ----
            ones_sb = small.tile([1, B], F32, name="ones_sb", tag="ones_sb")
            nc.sync.dma_start(out=ones_sb, in_=ones1)
            subt_sb = small.tile([P, NCH, B], F32, name="subt_sb", tag="subt_sb")
            nc.sync.dma_start(out=subt_sb, in_=subt)
            wqtc_sb = small.tile([P, NCH, P], F32, name="wqtc_sb", tag="wqtc_sb")
            nc.sync.dma_start(out=wqtc_sb, in_=wqtc)
            wkce_sb = small.tile([P, 1028], F32, name="wkce_sb", tag="wkce_sb")
            wk_dma = nc.sync.dma_start(out=wkce_sb, in_=wkce)
            bq_sb = small.tile([1, P], F32, name="bq_sb", tag="bq_sb")
            nc.sync.dma_start(out=bq_sb, in_=bqc1)
            sel_sb = small.tile([BLOC, BLOC * P], F32, name="sel_sb", tag="sel_sb")
            nc.sync.dma_start(out=sel_sb, in_=sel4)
            id_sb = small.tile([B, B], F32, name="id_sb", tag="id_sb")
            id_dma = nc.sync.dma_start(out=id_sb, in_=id32)

            # ---- warmup collective: pay ncfw startup off the critical path ----
            warm_in = dram.tile([1, B], F32, name="w_in", tag="w_in")
            nc.sync.dma_start(out=warm_in, in_=ones_sb)
            warm_out = dram.tile([1, B], F32, name="w_out", tag="w_out")
            nc.gpsimd.collective_compute(
                "AllReduce",
                OP.add,
                replica_groups=[list(range(NCORES))],
                ins=[warm_in.opt()],
                outs=[warm_out.opt()],
            )

            # ---- Q[b, a'] for ALL batches over this core's a-slice ----
            psQ = psqp.tile([B, P], F32, name="psQ", tag="psq")
            for j in range(NCH):
                nc.tensor.matmul(
                    psQ,
                    subt_sb[:, j, :],
                    wqtc_sb[:, j, :],
                    start=(j == 0),
                    stop=False,
                )
            nc.tensor.matmul(psQ, ones_sb, bq_sb, start=False, stop=True)  # += bq
            Q_sb = small.tile([B, P], F32, name="Q_sb", tag="Q_sb")
            nc.scalar.copy(Q_sb, psQ)

            # ---- transpose -> QT[a', b] ----
            psT = psbp.tile([P, B], F32, name="psT", tag="psb")
            nc.tensor.transpose(psT, Q_sb, id_sb)
            QT = small.tile([P, B], F32, name="QT", tag="QT")
            nc.scalar.copy(QT, psT)

            # ---- partial qt (+ cb in col 1024) on 3 concurrent PE col groups ----
            psq = psqp.tile([96, 512], F32, name="psq", tag="psq")
            nc.tensor.matmul(
                psq[0:B, :], QT, wkce_sb[:, 0:512], tile_position=(0, 0)
            )
            nc.tensor.matmul(
                psq[32 : 32 + B, :],
                QT,
                wkce_sb[:, 512:1024],
                tile_position=(0, 32),
            )
            nc.tensor.matmul(
                psq[64 : 64 + B, 0:4], QT, wkce_sb[:, 1024:1028], tile_position=(0, 64)
            )
            qtp_sb = small.tile([B, 1028], F32, name="qtp_sb", tag="qtp_sb")
            nc.scalar.mul(qtp_sb[:, 0:512], psq[0:B, :], SCALE)
            nc.scalar.mul(qtp_sb[:, 512:1024], psq[32 : 32 + B, :], SCALE)
            nc.scalar.mul(qtp_sb[:, 1024:1028], psq[64 : 64 + B, 0:4], SCALE)

            # ---- 8-core ReduceScatter: core c receives qt rows [4c:4c+4] ----
            bounce_in = dram.tile([B, 1028], F32, name="cc_in", tag="cc_in")
            nc.scalar.dma_start(out=bounce_in, in_=qtp_sb)
            rs_out = dram.tile([BLOC, 1028], F32, name="cc_out", tag="cc_out")
            nc.gpsimd.collective_compute(
                "ReduceScatter",
                OP.add,
                replica_groups=[list(range(NCORES))],
                ins=[bounce_in.opt()],
                outs=[rs_out.opt()],
            )
            qt4 = small.tile([BLOC, 1028], F32, name="qt4", tag="qt4")
            nc.scalar.dma_start(out=qt4, in_=rs_out)

            # ---- broadcast each batch's qt row to 128 partitions (PE) ----
            qb_all = small.tile([P, BLOC, T + 1], F32, name="qb_all", tag="qb_all")
            for b in range(BLOC):
                psb = psbp.tile([P, 512], F32, name="psb", tag="psb")
                nc.tensor.matmul(psb, sel_sb[:, b * P : (b + 1) * P], qt4[:, 0:512])
                nc.scalar.copy(qb_all[:, b, 0:512], psb)
                psb2 = psbp.tile([P, 512], F32, name="psb2", tag="psb")
                nc.tensor.matmul(
                    psb2, sel_sb[:, b * P : (b + 1) * P], qt4[:, 512:1024]
                )
                nc.scalar.copy(qb_all[:, b, 512:1024], psb2)
                psbc = psqp.tile([P, 4], F32, name="psbc", tag="psq")
                nc.tensor.matmul(
                    psbc, sel_sb[:, b * P : (b + 1) * P], qt4[:, 1024:1028]
                )
                nc.scalar.copy(qb_all[:, b, T : T + 1], psbc[:, 0:1])
            qb_t = [qb_all[:, b, :] for b in range(BLOC)]

            # ---- main stream: e = (r . qt) + cb ; ww += e^T-weighted rows ----
            rt_dmas = []
            for b in range(BLOC):
                ps_ww = pswwp.tile([33, T], F32, name="ps_ww", tag="ps_ww")
                for ti in range(NTI):
                    rt = rp.tile([P, LTILE, T], F32, name="rt", tag="rt")
                    d = nc.sync.dma_start(
                        out=rt,
                        in_=rev[
                            b, ti * ROWS_PER_TILE : (ti + 1) * ROWS_PER_TILE, :
                        ].rearrange("(p f) t -> p f t", f=LTILE),
                    )
                    if len(rt_dmas) == 0:
                        add_dep_helper(
                            d.ins, wk_dma.ins, reason="weights before rt stream"
                        )
                    elif len(rt_dmas) == 1:
                        add_dep_helper(
                            d.ins, id_dma.ins, reason="small loads before rt stream"
                        )
                    elif len(rt_dmas) >= 4:
                        add_dep_helper(
                            d.ins, rt_dmas[-4].ins, reason="stagger rt stream"
                        )
                    rt_dmas.append(d)
                    e_raw = ep.tile([P, LTILE], F32, name="e_raw", tag="e_raw")
                    e_t = ep.tile([P, LTILE], F32, name="e_t", tag="e_t")
                    for i in range(LTILE):
                        scr = scrp.tile([P, T], F32, name="scr", tag="scr")
                        nc.vector.scalar_tensor_tensor(
                            out=scr,
                            in0=rt[:, i, :],
                            scalar=1.0,
                            in1=qb_t[b][:, 0:T],
                            op0=OP.bypass,
                            op1=OP.mult,
                            accum_out=e_raw[:, i : i + 1],
                        )
                        nc.scalar.activation(
                            e_t[:, i : i + 1],
                            e_raw[:, i : i + 1],
                            FT.Identity,
                            bias=qb_t[b][:, T : T + 1],
                        )
                        for h in range(2):
                            nc.tensor.matmul(
                                ps_ww[
                                    32 * h : 32 * h + 1, h * 512 : (h + 1) * 512
                                ],
                                e_t[:, i : i + 1],
                                rt[:, i, h * 512 : (h + 1) * 512],
                                start=(ti == 0 and i == 0),
                                stop=(ti == NTI - 1 and i == LTILE - 1),
                                tile_position=(0, 32 * h),
                            )
                ww_sb = wwp.tile([33, T], F32, name="ww_sb", tag="ww_sb")
                nc.scalar.copy(ww_sb[0:1, 0:512], ps_ww[0:1, 0:512])
                nc.scalar.copy(ww_sb[32:33, 512:1024], ps_ww[32:33, 512:1024])
                nc.scalar.dma_start(out=out[b : b + 1, 0:512], in_=ww_sb[0:1, 0:512])
                nc.scalar.dma_start(
                    out=out[b : b + 1, 512:1024], in_=ww_sb[32:33, 512:1024]
                )

    nc.compile()
    return nc


_NC = None


def _get_nc():
    global _NC
    if _NC is None:
        _NC = _build()
    return _NC


def _in_maps(submitter_emb, reviewer_emb, Wq, bq, Wk, bk):
    submitter_emb = np.ascontiguousarray(submitter_emb, dtype=np.float32)
    reviewer_emb = np.ascontiguousarray(reviewer_emb, dtype=np.float32)
    Wq = np.asarray(Wq, dtype=np.float32)
    Wk = np.asarray(Wk, dtype=np.float32)
    bq = np.asarray(bq, dtype=np.float32)
    bk = np.asarray(bk, dtype=np.float32)

    subt = np.ascontiguousarray(submitter_emb.T.reshape(NCH, P, B).transpose(1, 0, 2))
    ones1 = np.ones((1, B), dtype=np.float32)
    sel4 = np.zeros((BLOC, BLOC * P), dtype=np.float32)
    for b_ in range(BLOC):
        sel4[b_, b_ * P : (b_ + 1) * P] = 1.0
    id32 = np.eye(B, dtype=np.float32)

    in_maps = []
    for core in range(NCORES):
        lo, hi = core * BLOC, (core + 1) * BLOC
        alo, ahi = core * P, (core + 1) * P
        wqtc = np.ascontiguousarray(
            Wq[alo:ahi, :].T.reshape(NCH, P, P).transpose(1, 0, 2)
        )
        wkce = np.zeros((P, 1028), dtype=np.float32)
        wkce[:, 0:1024] = Wk[alo:ahi, :]
        wkce[:, 1024] = bk[alo:ahi]
        in_maps.append(
            {
                "rev": reviewer_emb[lo:hi],
                "subt": subt,
                "wqtc": wqtc,
                "wkce": wkce,
                "ones1": ones1,
                "bqc1": np.ascontiguousarray(bq[alo:ahi][None, :]),
                "sel4": sel4,
                "id32": id32,
            }
        )
    return in_maps


def kernel(
    submitter_emb: np.ndarray,
    reviewer_emb: np.ndarray,
    Wq: np.ndarray,
    bq: np.ndarray,
    Wk: np.ndarray,
    bk: np.ndarray,
) -> np.ndarray:
    nc = _get_nc()
    in_maps = _in_maps(submitter_emb, reviewer_emb, Wq, bq, Wk, bk)
    res = run_bass_kernel_spmd(nc, in_maps, core_ids=list(range(NCORES)))
    return np.concatenate([res.results[c]["out"] for c in range(NCORES)], axis=0)


# revision 7
# speedup vs baseline: 1.5599x; 1.5599x over previous
"""Trainium2 Bass kernel for nn_Attention_Module (submitter/reviewer attention pooling).

Reference math:
    Q  = submitter_emb @ Wq.T + bq                      [B, A]
    K  = einsum('blt,at->bla', reviewer_emb, Wk) + bk   [B, L, A]
    e  = einsum('ba,bla->bl', Q, K) / sqrt(A)           [B, L]
    ww = einsum('bl,blt->bt', e, reviewer_emb)          [B, T]

Algebraic rewrite used here (exact, just reassociation):
    qt[b, t] = sum_a Q[b, a] * Wk[a, t]        (= Q @ Wk,   [B, T], tiny)
    cb[b]    = sum_a Q[b, a] * bk[a]           (scalar per batch)
    e[b, l]  = (reviewer[b, l, :] . qt[b, :] + cb[b]) / sqrt(A)
    ww[b, t] = sum_l e[b, l] * reviewer[b, l, t]

This collapses the 137-GFLOP K matmul into a single streaming pass over
reviewer_emb: one fused DVE multiply+reduce (scalar_tensor_tensor with
accum_out) per tile for e, and PE matmuls with e as the stationary
operand (lhsT [128,1]) streaming the reviewer tile as the moving
operand, accumulating ww as a [1, 1024] PSUM row.

Sharding: data-parallel over batch B=32 -> 4 batches per core x 8 cores.
Weights replicated.  No cross-core communication; host concatenates.

Host-side prep is layout-only (transpose/reshape of inputs for DMA
efficiency); all input-dependent arithmetic runs on device in fp32.
"""

import numpy as np

import concourse.bass as bass
import concourse.bacc as bacc
import concourse.tile as tile
from concourse.tile_rust import add_dep_helper
from concourse import mybir
from concourse.bass_utils import run_bass_kernel_spmd

# Problem shapes (hardcoded per contract)
B, L, T, A = 32, 2048, 1024, 1024
NCORES = 8
BLOC = B // NCORES          # 4 batches per core
P = 128                     # partitions
NCH = T // P                # 8 chunks of 128 along T/A
LTILE = 4                   # reviewer rows per partition per DMA tile
ROWS_PER_TILE = P * LTILE   # 512 rows -> 2 MiB per DMA
NTI = L // ROWS_PER_TILE    # 4 DMA tiles per batch
SCALE = 1.0 / float(np.sqrt(A))

F32 = mybir.dt.float32
FT = mybir.ActivationFunctionType
OP = mybir.AluOpType


def _build():
    nc = bacc.Bacc("TRN2", target_bir_lowering=False, debug=False, num_devices=NCORES)

    # subt: submitter slice pre-tiled to [128, NCH, BLOC] (t-major on partitions)
    subt = nc.dram_tensor("subt", [P, NCH, BLOC], F32, kind="ExternalInput").ap()
    rev = nc.dram_tensor("rev", [BLOC, L, T], F32, kind="ExternalInput").ap()
    # wqt: Wq.T, [T, A] row-major
    wqt = nc.dram_tensor("wqt", [T, A], F32, kind="ExternalInput").ap()
    # bqc/bkc: biases chunked [128, NCH] with element (p, c) = bias[128c + p]
    bqc = nc.dram_tensor("bqc", [P, NCH], F32, kind="ExternalInput").ap()
    wk = nc.dram_tensor("wk", [A, T], F32, kind="ExternalInput").ap()
    bkc = nc.dram_tensor("bkc", [P, NCH], F32, kind="ExternalInput").ap()
    ident = nc.dram_tensor("ident", [BLOC, BLOC], F32, kind="ExternalInput").ap()
    # sel: one-hot selector for the PE row-broadcast of qt. Rows 0:BLOC and
    # 32:32+BLOC both hold eye(BLOC) replicated 128 wide: sel[k, 128b+m] = (k==b)
    sel = nc.dram_tensor("sel", [36, BLOC * P], F32, kind="ExternalInput").ap()
    out = nc.dram_tensor("out", [BLOC, T], F32, kind="ExternalOutput").ap()

    with tile.TileContext(nc) as tc:
        with (
            tc.tile_pool(name="small", bufs=1) as small,
            tc.tile_pool(name="wqtp", bufs=1) as wqtp,
            tc.tile_pool(name="wkp", bufs=1) as wkp,
            tc.tile_pool(name="qb", bufs=1) as qbp,
            tc.tile_pool(name="rp", bufs=6) as rp,
            tc.tile_pool(name="scr", bufs=3) as scrp,
            tc.tile_pool(name="ep", bufs=4) as ep,
            tc.tile_pool(name="wwp", bufs=2) as wwp,
            tc.tile_pool(name="dram", bufs=1, space="DRAM") as dram,
            tc.tile_pool(name="pstr", bufs=2, space="PSUM") as pstrp,
            tc.tile_pool(name="psq", bufs=1, space="PSUM") as psqp,
            tc.tile_pool(name="psww", bufs=2, space="PSUM") as pswwp,
        ):
            # ---- small loads (already laid out by host) ----
            subt_sb = small.tile([P, NCH, BLOC], F32, name="subt_sb", tag="subt_sb")
            nc.sync.dma_start(out=subt_sb, in_=subt)
            bq_sb = small.tile([P, NCH], F32, name="bq_sb", tag="bq_sb")
            nc.sync.dma_start(out=bq_sb, in_=bqc)
            bk_sb = small.tile([P, NCH], F32, name="bk_sb", tag="bk_sb")
            nc.sync.dma_start(out=bk_sb, in_=bkc)
            id_sb = small.tile([BLOC, BLOC], F32, name="id_sb", tag="id_sb")
            nc.sync.dma_start(out=id_sb, in_=ident)
            sel_sb = small.tile([36, BLOC * P], F32, name="sel_sb", tag="sel_sb")
            nc.sync.dma_start(out=sel_sb, in_=sel)

            # ---- weight loads: wqT[j] = [128 t, 1024 a], wk[i] = [128 a, 1024 t] ----
            wqT = [
                wqtp.tile([P, A], F32, name=f"wqT{j}", tag=f"wqT{j}")
                for j in range(NCH)
            ]
            for j in range(NCH):
                nc.sync.dma_start(out=wqT[j], in_=wqt[j * P : (j + 1) * P, :])
            wk_sb = [
                wkp.tile([P, T], F32, name=f"wk{i}", tag=f"wk{i}") for i in range(NCH)
            ]
            wk_dmas = []
            for i in range(NCH):
                wk_dmas.append(
                    nc.sync.dma_start(out=wk_sb[i], in_=wk[i * P : (i + 1) * P, :])
                )

            # ---- Q[b, a] = s @ Wq.T : Wq.T streams as the moving operand,
            #      so these matmuls overlap the weight-DMA wave chunk by chunk.
            psQ = psqp.tile([36, A], F32, name="psQ", tag="psq")[0:BLOC, :]
            for j in range(NCH):
                for h in range(2):
                    nc.tensor.matmul(
                        psQ[:, h * 512 : (h + 1) * 512],
                        subt_sb[:, j, :],
                        wqT[j][:, h * 512 : (h + 1) * 512],
                        start=(j == 0),
                        stop=(j == NCH - 1),
                    )
            Q_sb = small.tile([BLOC, A], F32, name="Q_sb", tag="Q_sb")
            nc.scalar.copy(Q_sb, psQ)

            # ---- QT chunks [128 a, BLOC] via tiny PE transposes; add bq here ----
            QT = small.tile([P, NCH, BLOC], F32, name="QT", tag="QT")
            for i in range(NCH):
                pstr = pstrp.tile([P, BLOC], F32, name="pstr", tag="pstr")
                nc.tensor.transpose(pstr, Q_sb[:, i * P : (i + 1) * P], id_sb)
                nc.scalar.activation(
                    QT[:, i, :], pstr, FT.Identity, bias=bq_sb[:, i : i + 1]
                )

            # ---- qt[b, t] = Q @ Wk on two concurrent PE col-groups;
            #      cb = Q . bk in a separate PSUM bank ----
            psq = psqp.tile([36, T], F32, name="psq", tag="psq")
            cb_ps = pstrp.tile([BLOC, 1], F32, name="cb_ps", tag="pstr")
            for i in range(NCH):
                nc.tensor.matmul(
                    psq[0:BLOC, 0:512],
                    QT[:, i, :],
                    wk_sb[i][:, 0:512],
                    start=(i == 0),
                    stop=(i == NCH - 1),
                    tile_position=(0, 0),
                )
                nc.tensor.matmul(
                    psq[32 : 32 + BLOC, 512:1024],
                    QT[:, i, :],
                    wk_sb[i][:, 512:1024],
                    start=(i == 0),
                    stop=(i == NCH - 1),
                    tile_position=(0, 32),
                )
                nc.tensor.matmul(
                    cb_ps,
                    QT[:, i, :],
                    bk_sb[:, i : i + 1],
                    start=(i == 0),
                    stop=(i == NCH - 1),
                )
            # fold 1/sqrt(A) here; halves sit on partitions 0-3 / 32-35 and
            # get re-joined (and row-broadcast) by the selector matmuls below
            qts0 = small.tile([BLOC, 512], F32, name="qts0", tag="qts0")
            nc.scalar.mul(qts0, psq[0:BLOC, 0:512], SCALE)
            qts1 = small.tile([36, 512], F32, name="qts1", tag="qts1")
            nc.scalar.mul(qts1[32 : 32 + BLOC, :], psq[32 : 32 + BLOC, 512:1024], SCALE)
            qtc = small.tile([BLOC, 1], F32, name="qtc", tag="qtc")
            nc.scalar.mul(qtc, cb_ps, SCALE)

            # ---- broadcast qt rows to 128 partitions on the PE: one-hot
            #      selector as stationary, qt pieces as moving operand ----
            qb_all = qbp.tile([P, BLOC, T + 1], F32, name="qb_all", tag="qb_all")
            for b in range(BLOC):
                qb_ps = pswwp.tile([P, 512], F32, name="qb_ps", tag="ps_ww")
                nc.tensor.matmul(
                    qb_ps, sel_sb[0:BLOC, b * P : (b + 1) * P], qts0
                )
                nc.scalar.copy(qb_all[:, b, 0:512], qb_ps)
                qb_ps2 = pswwp.tile([P, 512], F32, name="qb_ps2", tag="ps_ww")
                nc.tensor.matmul(
                    qb_ps2,
                    sel_sb[32 : 32 + BLOC, b * P : (b + 1) * P],
                    qts1[32 : 32 + BLOC, :],
                )
                nc.scalar.copy(qb_all[:, b, 512:1024], qb_ps2)
                qb_psc = pstrp.tile([P, 1], F32, name="qb_psc", tag="pstr")
                nc.tensor.matmul(
                    qb_psc, sel_sb[0:BLOC, b * P : (b + 1) * P], qtc
                )
                nc.scalar.copy(qb_all[:, b, 1024 : T + 1], qb_psc)
            qb_t = [qb_all[:, b, :] for b in range(BLOC)]

            # ---- main stream: e = (r . qt) + cb ; ww += e.T-weighted rows ----
            rt_dmas = []  # chain reviewer DMAs depth-3 so they complete in
            # order (unchained, the round-robin queues finish the whole
            # first wave together, gating the first compute tile)
            for b in range(BLOC):
                # ww halves accumulate on two concurrent PE col-groups:
                # half 0 -> psum row 0 cols 0:512 (bank 0), half 1 -> psum
                # row 32 cols 512:1024 (bank 1); one start/stop pair each.
                ps_ww = pswwp.tile([33, T], F32, name="ps_ww", tag="ps_ww")
                for ti in range(NTI):
                    rt = rp.tile([P, LTILE, T], F32, name="rt", tag="rt")
                    d = nc.sync.dma_start(
                        out=rt,
                        in_=rev[
                            b, ti * ROWS_PER_TILE : (ti + 1) * ROWS_PER_TILE, :
                        ].rearrange("(p f) t -> p f t", f=LTILE),
                    )
                    if len(rt_dmas) < 4:
                        add_dep_helper(
                            d.ins,
                            wk_dmas[4 + len(rt_dmas)].ins,
                            reason="weights drain before rt stream",
                        )
                    else:
                        add_dep_helper(
                            d.ins, rt_dmas[-4].ins, reason="stagger rt stream"
                        )
                    rt_dmas.append(d)
                    e_raw = ep.tile([P, LTILE], F32, name="e_raw", tag="e_raw")
                    e_t = ep.tile([P, LTILE], F32, name="e_t", tag="e_t")
                    for i in range(LTILE):
                        # fused multiply + free-dim reduce on DVE:
                        # scr = r * qt_bcast ; e_raw = sum(scr)
                        scr = scrp.tile([P, T], F32, name="scr", tag="scr")
                        nc.vector.scalar_tensor_tensor(
                            out=scr,
                            in0=rt[:, i, :],
                            scalar=1.0,
                            in1=qb_t[b][:, 0:T],
                            op0=OP.bypass,
                            op1=OP.mult,
                            accum_out=e_raw[:, i : i + 1],
                        )
                        # e = e_raw + cb on ScalarE (cb pre-scaled by 1/sqrt(A))
                        nc.scalar.activation(
                            e_t[:, i : i + 1],
                            e_raw[:, i : i + 1],
                            FT.Identity,
                            bias=qb_t[b][:, T : T + 1],
                        )
                        # ww[0, :] += e_slice.T @ r_slice  (e stationary,
                        # reviewer tile streams as the moving operand)
                        for h in range(2):
                            nc.tensor.matmul(
                                ps_ww[
                                    32 * h : 32 * h + 1, h * 512 : (h + 1) * 512
                                ],
                                e_t[:, i : i + 1],
                                rt[:, i, h * 512 : (h + 1) * 512],
                                start=(ti == 0 and i == 0),
                                stop=(ti == NTI - 1 and i == LTILE - 1),
                                tile_position=(0, 32 * h),
                            )
                ww_sb = wwp.tile([33, T], F32, name="ww_sb", tag="ww_sb")
                nc.scalar.copy(ww_sb[0:1, 0:512], ps_ww[0:1, 0:512])
                nc.scalar.copy(ww_sb[32:33, 512:1024], ps_ww[32:33, 512:1024])
                nc.scalar.dma_start(out=out[b : b + 1, 0:512], in_=ww_sb[0:1, 0:512])
                nc.scalar.dma_start(
                    out=out[b : b + 1, 512:1024], in_=ww_sb[32:33, 512:1024]
                )

    nc.compile()
    return nc


_NC = None


def _get_nc():
    global _NC
    if _NC is None:
        _NC = _build()
    return _NC


def _in_maps(submitter_emb, reviewer_emb, Wq, bq, Wk, bk):
    submitter_emb = np.ascontiguousarray(submitter_emb, dtype=np.float32)
    reviewer_emb = np.ascontiguousarray(reviewer_emb, dtype=np.float32)
    # host-side layout prep (no arithmetic): transposes / chunking for DMA
    wqt_np = np.ascontiguousarray(np.asarray(Wq, dtype=np.float32).T)
    wk_np = np.ascontiguousarray(Wk, dtype=np.float32)
    bqc = np.ascontiguousarray(np.asarray(bq, dtype=np.float32).reshape(NCH, P).T)
    bkc = np.ascontiguousarray(np.asarray(bk, dtype=np.float32).reshape(NCH, P).T)
    ident = np.eye(BLOC, dtype=np.float32)
    sel = np.zeros((36, BLOC * P), dtype=np.float32)
    for b_ in range(BLOC):
        sel[b_, b_ * P : (b_ + 1) * P] = 1.0
        sel[32 + b_, b_ * P : (b_ + 1) * P] = 1.0

    in_maps = []
    for core in range(NCORES):
        lo, hi = core * BLOC, (core + 1) * BLOC
        # [BLOC, T] -> [128 p, NCH c, BLOC b] with t = 128*c + p
        subt = np.ascontiguousarray(
            submitter_emb[lo:hi].T.reshape(NCH, P, BLOC).transpose(1, 0, 2)
        )
        in_maps.append(
            {
                "subt": subt,
                "rev": reviewer_emb[lo:hi],
                "wqt": wqt_np,
                "bqc": bqc,
                "wk": wk_np,
                "bkc": bkc,
                "ident": ident,
                "sel": sel,
            }
        )
    return in_maps


def kernel(
    submitter_emb: np.ndarray,
    reviewer_emb: np.ndarray,
    Wq: np.ndarray,
    bq: np.ndarray,
    Wk: np.ndarray,
    bk: np.ndarray,
) -> np.ndarray:
    nc = _get_nc()
    in_maps = _in_maps(submitter_emb, reviewer_emb, Wq, bq, Wk, bk)
    res = run_bass_kernel_spmd(nc, in_maps, core_ids=list(range(NCORES)))
    return np.concatenate([res.results[c]["out"] for c in range(NCORES)], axis=0)

